# revision 22
# baseline (speedup 1.0000x reference)
"""ChildSumTreeLSTM on 8 trn2 NeuronCores — fused-level rewrite, v2.

Tree: reversed complete 4-ary heap (id = N-1-heap; heap j's children 4j+1..4j+4).
The 64 depth-3 subtrees are assigned round-robin (core c gets heaps 21+8s+c,
s=0..7) so real leaf work balances: only subtrees with heap<=63 have depth-6
leaves, so every core has at most 6 leafy subtrees -> leaf level = 384 slots.
Per-core slot array: leaf 384, L2 128, L1 32, L0 8, T2 16, T1 4, T0 1, pad 3
= 576 slots.

Layout: mem dim (512 = 4 m-tiles of 128) on partitions, nodes on free dim; all
state fused as single [128, 4m*n] tiles, gate order (i, u, o, f).  One merged
Wx GEMM computes leaf preacts and XLev (X of all level slots) per (k, mc) pair;
leaf gates activate straight from PSUM (per-m bias via ACT bias column); a
matmul's PSUM output must never cross a 2KB bank boundary (silent corruption).
Level steps add XLev into the Ws-GEMM psum with per-gate DVE tensor_tensor adds
(stride-0 broadcast for the f-gate), and the forget path is pipelined per
m-tile under the Ws GEMM.  The 64 subtree roots are AllGathered in f32
node-major form (PE transposes); the round-robin assignment is undone inside
the PSUM->SBUF copy access patterns.  A tiny dummy AllGather keeps the CC
stream busy so the real collective starts ~2us after its data instead of
~11.5us.  Every core then computes the 21-node top tree redundantly.
"""

import os
import sys

sys.path.insert(0, "/opt/trn_rl_repo")

import numpy as np

import concourse.bass as bass
import concourse.bacc as bacc
import concourse.mybir as mybir
import concourse.tile as tile
from concourse.bass_utils import run_bass_kernel_spmd

F32 = mybir.dt.float32
F16 = mybir.dt.float16
AF = mybir.ActivationFunctionType
ALU = mybir.AluOpType
AX = mybir.AxisListType

N = 4096
MEM = 512
IN_DIM = 512
NCORES = 8
P = 128
KT = 4

NL3 = 384                # leaf slots (6 leafy subtrees x 64)
NSLOT = 576
XLEV_BASE = 384          # XLev covers slots [384, 573)
XLEV_N = 189
W573 = NL3 + XLEV_N      # merged leaf+XLev GEMM width per mc

LAST_RESULT = None


def _core_heaps(c):
    # subtree s of core c is rooted at heap 21 + 8*s + c (round-robin)
    t = [21 + 8 * s + c for s in range(8)]
    heaps = []
    for s in range(6):
        heaps += [64 * t[s] + 21 + a for a in range(64)]  # leaf (depth 6)
    for s in range(8):
        heaps += [16 * t[s] + 5 + a for a in range(16)]  # L2 (depth 5)
    for s in range(8):
        heaps += [4 * t[s] + 1 + a for a in range(4)]  # L1 (depth 4)
    for s in range(8):
        heaps += [t[s]]  # L0 (depth 3)
    heaps += list(range(5, 21)) + list(range(1, 5)) + [0]  # T2, T1, T0
    heaps += [-1, -1, -1]  # pad to 576
    return np.array(heaps, dtype=np.int64)


def _build_program():
    nc = bacc.Bacc("TRN2", target_bir_lowering=False, debug=False)

    xin_d = nc.dram_tensor("xin", [P, KT * NSLOT], F16, kind="ExternalInput")
    wxiu_d = nc.dram_tensor("wxiu", [P, KT * 1024], F16, kind="ExternalInput")
    wxof_d = nc.dram_tensor("wxof", [P, KT * 1024], F16, kind="ExternalInput")
    ws_d = nc.dram_tensor("ws", [P, KT * 3 * MEM], F16, kind="ExternalInput")
    wf_d = nc.dram_tensor("wf", [P, KT * MEM], F16, kind="ExternalInput")
    bcol_d = nc.dram_tensor("bcol", [P, 16], F32, kind="ExternalInput")
    cm_d = nc.dram_tensor("cmask", [P, NL3], F16, kind="ExternalInput")
    id_d = nc.dram_tensor("ident", [P, P], F32, kind="ExternalInput")
    out_d = nc.dram_tensor("out", [1, MEM], F32, kind="ExternalOutput")
    DBG = bool(os.environ.get("KERNEL_DEBUG"))
    if DBG:
        xlev_dbg = nc.dram_tensor("xlev_dbg", [P, 16 * XLEV_N], F16, kind="ExternalOutput")
        h3_dbg = nc.dram_tensor("h3_dbg", [P, 2048], F16, kind="ExternalOutput")
        c3_dbg = nc.dram_tensor("c3_dbg", [P, 2048], F16, kind="ExternalOutput")
        h2_dbg = nc.dram_tensor("h2_dbg", [P, 512], F16, kind="ExternalOutput")
        f2_dbg = nc.dram_tensor("f2_dbg", [P, 2048], F16, kind="ExternalOutput")
        pre2_dbg = nc.dram_tensor("pre2_dbg", [P, 1536], F16, kind="ExternalOutput")
        h0_dbg = nc.dram_tensor("h0_dbg", [P, 32], F32, kind="ExternalOutput")
        hnm_dbg = nc.dram_tensor("hnm_dbg", [64, MEM], F32, kind="ExternalOutput")
        h64_dbg = nc.dram_tensor("h64_dbg", [P, 256], F16, kind="ExternalOutput")
        ht2_dbg = nc.dram_tensor("ht2_dbg", [P, 64], F16, kind="ExternalOutput")
    warmcc_d = nc.dram_tensor("warmcc", [1, 16], F32)
    gathw_d = nc.dram_tensor("gathw", [8, 16], F32, addr_space="Shared")
    contrib_d = nc.dram_tensor("contrib", [8, 2 * MEM], F32)
    gath_d = nc.dram_tensor("gath", [64, 2 * MEM], F32, addr_space="Shared")

    with tile.TileContext(nc) as tc:
        with (
            tc.tile_pool(name="wpool", bufs=1) as wpool,
            tc.tile_pool(name="spool", bufs=1) as spool,
            tc.tile_pool(name="psp", bufs=1, space="PSUM") as psp,
        ):
            ps_cnt = [0]

            def ps():
                t = psp.tile([P, 2048], F32, name="t", tag=f"ps{ps_cnt[0] % 2}")
                ps_cnt[0] += 1
                return t

            # ---- loads: one big k-major DMA per tensor ----
            xin_t = wpool.tile([P, KT * NSLOT], F16, name="t", tag="xin")
            wxiu_t = wpool.tile([P, KT * 1024], F16, name="t", tag="wxiu")
            wxof_t = wpool.tile([P, KT * 1024], F16, name="t", tag="wxof")
            ws_t = wpool.tile([P, KT * 3 * MEM], F16, name="t", tag="ws")
            wf_t = wpool.tile([P, KT * MEM], F16, name="t", tag="wf")
            bcol_s = wpool.tile([P, 16], F32, name="t", tag="bcol")
            cm_s = wpool.tile([P, NL3], F16, name="t", tag="cm")
            id_s = wpool.tile([P, P], F32, name="t", tag="id")

            for k in range(KT):
                nc.sync.dma_start(
                    xin_t[:, k * NSLOT:(k + 1) * NSLOT],
                    xin_d[:, k * NSLOT:(k + 1) * NSLOT])
                nc.sync.dma_start(
                    wxiu_t[:, k * 1024:(k + 1) * 1024],
                    wxiu_d[:, k * 1024:(k + 1) * 1024])
            nc.sync.dma_start(bcol_s[:], bcol_d[:])
            nc.sync.dma_start(warmcc_d[:], bcol_s[0:1, 0:16])
            nc.gpsimd.collective_compute(
                "AllGather", ALU.bypass,
                replica_groups=[list(range(NCORES))],
                ins=[warmcc_d[:]],
                outs=[gathw_d[:]],
            )
            nc.sync.dma_start(cm_s[:], cm_d[:])
            nc.sync.dma_start(id_s[:], id_d[:])
            nc.sync.dma_start(wxof_t[:], wxof_d[:])
            nc.sync.dma_start(wf_t[:], wf_d[:])
            nc.sync.dma_start(ws_t[:], ws_d[:])

            def wx_sl(k, mc, width=P):
                # wx lhsT tile for (k, mc): gates i,u in wxiu, o,f in wxof
                if mc < 8:
                    return wxiu_t[:, k * 1024 + mc * P:k * 1024 + mc * P + width]
                return wxof_t[:, k * 1024 + (mc - 8) * P:k * 1024 + (mc - 8) * P + width]

            XLev = wpool.tile([P, 16 * XLEV_N], F16, name="t", tag="xlev")

            # ========== leaf + XLev: one merged Wx GEMM ==========
            H3 = spool.tile([P, 4 * 512], F16, name="t", tag="H3")
            C3 = spool.tile([P, 4 * 512], F16, name="t", tag="C3")
            nc.vector.memset(H3[:], 0.0)
            nc.vector.memset(C3[:], 0.0)

            # gates i, u, o: psum [128, 2*573] per (gate, m-pair)
            Gt = {}
            for g in range(3):
                Gt[g] = spool.tile([P, 4 * NL3], F16, name="t", tag=f"G{g}")
            for g, fn in ((0, AF.Sigmoid), (1, AF.Tanh), (2, AF.Sigmoid)):
                for mp in (0, 1):  # m pairs (0,1) and (2,3)
                    pst = ps()
                    for mi in (0, 1):
                        m = 2 * mp + mi
                        mc = g * 4 + m
                        # matmul psum output must not cross a 1024-f32 (4KB)
                        # boundary: leaf chunk at mi*512, XLev chunk in the
                        # upper half at 1024 + 192*mi
                        for k in range(KT):
                            nc.tensor.matmul(
                                pst[:, mi * 512:mi * 512 + NL3],
                                wx_sl(k, mc),
                                xin_t[:, k * NSLOT:k * NSLOT + NL3],
                                start=(k == 0), stop=(k == KT - 1),
                            )
                            nc.tensor.matmul(
                                pst[:, 1024 + 192 * mi:1024 + 192 * mi + XLEV_N],
                                wx_sl(k, mc),
                                xin_t[:, k * NSLOT + NL3:k * NSLOT + W573],
                                start=(k == 0), stop=(k == KT - 1),
                            )
                    for mi in (0, 1):
                        m = 2 * mp + mi
                        mc = g * 4 + m
                        # leaf part: activate straight from psum (+bias)
                        nc.scalar.activation(
                            Gt[g][:, m * NL3:(m + 1) * NL3],
                            pst[:, mi * 512:mi * 512 + NL3],
                            fn, bias=bcol_s[:, mc:mc + 1])
                        # XLev part: psum + bias -> fp16 (split scalar/vector)
                        xsl = slice(mc * XLEV_N, (mc + 1) * XLEV_N)
                        psl = pst[:, 1024 + 192 * mi:1024 + 192 * mi + XLEV_N]
                        if mi == 0:
                            nc.vector.tensor_scalar_add(
                                XLev[:, xsl], psl, bcol_s[:, mc:mc + 1])
                        else:
                            nc.scalar.add(XLev[:, xsl], psl, bcol_s[:, mc:mc + 1])
            # f gate: XLev only
            pst = ps()
            for m in range(4):
                mc = 12 + m
                # one 189-wide region per 512-f32 psum bank (no bank crossing)
                osl = slice(m * 512, m * 512 + XLEV_N)
                for k in range(KT):
                    nc.tensor.matmul(
                        pst[:, osl],
                        wx_sl(k, mc),
                        xin_t[:, k * NSLOT + XLEV_BASE:k * NSLOT + XLEV_BASE + XLEV_N],
                        start=(k == 0), stop=(k == KT - 1),
                    )
                xsl = slice(mc * XLEV_N, (mc + 1) * XLEV_N)
                if m % 2 == 0:
                    nc.vector.tensor_scalar_add(
                        XLev[:, xsl], pst[:, osl], bcol_s[:, mc:mc + 1])
                else:
                    nc.scalar.add(XLev[:, xsl], pst[:, osl], bcol_s[:, mc:mc + 1])

            # leaf elementwise: c = (i*u)*mask, h = o*tanh(c)
            c3v = bass.AP(tensor=C3[:].tensor, offset=C3[:].offset,
                          ap=[list(C3[:].ap[0]), [512, 4], [1, NL3]])
            h3v = bass.AP(tensor=H3[:].tensor, offset=H3[:].offset,
                          ap=[list(H3[:].ap[0]), [512, 4], [1, NL3]])
            cmv = bass.AP(tensor=cm_s[:].tensor, offset=cm_s[:].offset,
                          ap=[list(cm_s[:].ap[0]), [0, 4], [1, NL3]])
            iu3 = spool.tile([P, 4 * NL3], F16, name="t", tag="iu3")
            th3 = spool.tile([P, 4 * NL3], F16, name="t", tag="th3")
            nc.vector.tensor_mul(iu3[:], Gt[0][:], Gt[1][:])
            nc.vector.tensor_mul(
                c3v, iu3[:].rearrange("p (m n) -> p m n", m=4), cmv)
            nc.scalar.activation(
                th3[:].rearrange("p (m n) -> p m n", m=4), c3v, AF.Tanh)
            nc.vector.tensor_mul(h3v,
                                 Gt[2][:].rearrange("p (m n) -> p m n", m=4),
                                 th3[:].rearrange("p (m n) -> p m n", m=4))

            _lvdbg = {}

            # ========== fused level step ==========
            def level_step(nm, n, soff, Hc, Cc, nch_stride, h_dtype=F16,
                           c_dtype=F16):
                """Hc/Cc: [128, 4m*nch_stride] tiles; children in cols [0, 4n)."""
                nch = 4 * n
                x_lo = soff - XLEV_BASE

                def cview(t, inner):
                    return bass.AP(
                        tensor=t[:].tensor, offset=t[:].offset,
                        ap=[list(t[:].ap[0]), [nch_stride, 4]] + inner)

                # child-h sum -> fp16
                chsf = spool.tile([P, nch], F32, name="t", tag=f"chsf{nm}")
                chs16 = spool.tile([P, nch], F16, name="t", tag=f"chs{nm}")
                nc.vector.tensor_reduce(
                    chsf[:].rearrange("p (m n) -> p m n", m=4),
                    cview(Hc, [[4, n], [1, 4]]),
                    axis=AX.X, op=ALU.add,
                )
                nc.vector.tensor_copy(chs16[:], chsf[:])

                # forget path: psF = Wf.T @ Hc ; pre_f = psF + XLevF ; sigmoid
                pf = ps()
                f16 = spool.tile([P, 4 * nch], F16, name="t", tag=f"f{nm}")
                for m in range(4):
                    osl = slice(m * nch, (m + 1) * nch)
                    for k in range(KT):
                        nc.tensor.matmul(
                            pf[:, osl],
                            wf_t[:, k * MEM + m * P:k * MEM + (m + 1) * P],
                            Hc[:, k * nch_stride:k * nch_stride + nch],
                            start=(k == 0), stop=(k == KT - 1),
                        )
                xfv = bass.AP(
                    tensor=XLev[:].tensor,
                    offset=XLev[:].offset + 12 * XLEV_N + x_lo,
                    ap=[list(XLev[:].ap[0]), [XLEV_N, 4], [1, n], [0, 4]])
                nc.vector.tensor_add(
                    f16[:].rearrange("p (m n g) -> p m n g", m=4, g=4),
                    pf[:, 0:4 * nch].rearrange("p (m n g) -> p m n g", m=4, g=4),
                    xfv)
                nc.scalar.activation(f16[:], f16[:], AF.Sigmoid)

                # iou: psum = Ws.T @ chs ; pre = psum + XLev ; activate
                piou = ps()
                for g in range(3):
                    for m in range(4):
                        osl = slice((g * 4 + m) * n, (g * 4 + m + 1) * n)
                        mc = g * 4 + m
                        for k in range(KT):
                            nc.tensor.matmul(
                                piou[:, osl],
                                ws_t[:, k * 1536 + mc * P:k * 1536 + (mc + 1) * P],
                                chs16[:, k * n:(k + 1) * n],
                                start=(k == 0), stop=(k == KT - 1),
                            )
                pre = spool.tile([P, 12 * n], F16, name="t", tag=f"pre{nm}")
                xv = bass.AP(
                    tensor=XLev[:].tensor, offset=XLev[:].offset + x_lo,
                    ap=[list(XLev[:].ap[0]), [XLEV_N, 12], [1, n]])
                nc.vector.tensor_add(
                    pre[:].rearrange("p (c n) -> p c n", c=12),
                    piou[:, 0:12 * n].rearrange("p (c n) -> p c n", c=12), xv)
                Gio = spool.tile([P, 2 * nch], F16, name="t", tag=f"Gio{nm}")
                Gu2 = spool.tile([P, nch], F16, name="t", tag=f"Gu2{nm}")
                iov = bass.AP(
                    tensor=pre[:].tensor, offset=pre[:].offset,
                    ap=[list(pre[:].ap[0]), [8 * n, 2], [1, nch]])
                nc.scalar.activation(
                    Gio[:].rearrange("p (a b) -> p a b", a=2), iov, AF.Sigmoid)
                nc.scalar.activation(Gu2[:], pre[:, nch:2 * nch], AF.Tanh)

                # fcc = sum_children f * c
                fcc = spool.tile([P, 4 * nch], F16, name="t", tag=f"fcc{nm}")
                fs = spool.tile([P, nch], F32, name="t", tag=f"fs{nm}")
                nc.vector.tensor_mul(
                    fcc[:].rearrange("p (m c) -> p m c", m=4),
                    f16[:].rearrange("p (m c) -> p m c", m=4),
                    cview(Cc, [[1, nch]]))
                nc.vector.tensor_reduce(
                    fs[:].rearrange("p (m n) -> p m n", m=4),
                    fcc[:].rearrange("p (m n g) -> p m n g", m=4, g=4),
                    axis=AX.X, op=ALU.add,
                )

                iu = spool.tile([P, nch], F16, name="t", tag=f"iu{nm}")
                Cp = spool.tile([P, nch], c_dtype, name="t", tag=f"C{nm}")
                thp = spool.tile([P, nch], F16, name="t", tag=f"th{nm}")
                Hp = spool.tile([P, nch], h_dtype, name="t", tag=f"H{nm}")
                nc.vector.tensor_mul(iu[:], Gio[:, 0:nch], Gu2[:])
                nc.vector.tensor_add(Cp[:], iu[:], fs[:])
                nc.scalar.activation(thp[:], Cp[:], AF.Tanh)
                nc.vector.tensor_mul(Hp[:], Gio[:, nch:2 * nch], thp[:])
                _lvdbg[nm] = (f16, pre)
                return Hp, Cp

            H2, C2 = level_step("L2", 128, 384, H3, C3, 512)
            H1, C1 = level_step("L1", 32, 512, H2, C2, 128)
            H0, C0 = level_step("L0", 8, 544, H1, C1, 32,
                                h_dtype=F32, c_dtype=F32)

            # ===== collective: roots -> node-major f32, AllGather =====
            PT = ps()  # [8, 1024]: h mem 0:512, c mem 512:1024
            for m in range(KT):
                nc.tensor.transpose(
                    PT[0:8, m * P:(m + 1) * P], H0[:, m * 8:(m + 1) * 8], id_s[:])
                nc.tensor.transpose(
                    PT[0:8, MEM + m * P:MEM + (m + 1) * P], C0[:, m * 8:(m + 1) * 8],
                    id_s[:])
            contribT = spool.tile([8, 2 * MEM], F32, name="t", tag="contribT")
            nc.scalar.copy(contribT[:], PT[0:8, 0:2 * MEM])
            nc.sync.dma_start(contrib_d[:], contribT[:])
            nc.gpsimd.collective_compute(
                "AllGather", ALU.bypass,
                replica_groups=[list(range(NCORES))],
                ins=[contrib_d[:]],
                outs=[gath_d[:]],
            )
            # reload in gather order (rows 8c+s); heap order restored in the
            # psum->sbuf copies below via permuted APs
            Hnm = spool.tile([64, MEM], F32, name="t", tag="Hnm")
            Cnm = spool.tile([64, MEM], F32, name="t", tag="Cnm")
            nc.sync.dma_start(Hnm[:], gath_d[:, 0:MEM])
            nc.sync.dma_start(Cnm[:], gath_d[:, MEM:2 * MEM])
            PT2 = ps()  # [128, 512]: H64 cols 0:256, C64 cols 256:512
            for m in range(KT):
                nc.tensor.transpose(
                    PT2[:, m * 64:(m + 1) * 64], Hnm[:, m * P:(m + 1) * P],
                    id_s[0:64, 0:64])
                nc.tensor.transpose(
                    PT2[:, 256 + m * 64:256 + (m + 1) * 64], Cnm[:, m * P:(m + 1) * P],
                    id_s[0:64, 0:64])
            H64 = spool.tile([P, 256], F16, name="t", tag="H64")
            C64 = spool.tile([P, 256], F16, name="t", tag="C64")
            # psum col m*64 + 8c+s  ->  sbuf col m*64 + 8s+c  (heap order)
            for base, dst in ((0, H64), (256, C64)):
                srcv = bass.AP(
                    tensor=PT2[:].tensor, offset=PT2[:].offset + base,
                    ap=[list(PT2[:].ap[0]), [64, 4], [1, 8], [8, 8]])
                dstv = dst[:].rearrange("p (m s c) -> p m s c", m=4, s=8)
                nc.scalar.copy(dstv, srcv)

            # ================= top tree =================
            HT2, CT2 = level_step("T2", 16, 552, H64, C64, 64)
            HT1, CT1 = level_step("T1", 4, 568, HT2, CT2, 16)
            HT0, _ = level_step("T0", 1, 572, HT1, CT1, 4, h_dtype=F32)

            if DBG:
                nc.sync.dma_start(xlev_dbg[:], XLev[:])
                nc.sync.dma_start(h3_dbg[:], H3[:])
                nc.sync.dma_start(c3_dbg[:], C3[:])
                nc.sync.dma_start(h2_dbg[:], H2[:])
                nc.sync.dma_start(f2_dbg[:], _lvdbg["L2"][0][:])
                nc.sync.dma_start(pre2_dbg[:], _lvdbg["L2"][1][:])
                nc.sync.dma_start(h0_dbg[:], H0[:])
                nc.sync.dma_start(hnm_dbg[:], Hnm[:])
                nc.sync.dma_start(h64_dbg[:], H64[:])
                nc.sync.dma_start(ht2_dbg[:], HT2[:])

            PT3 = ps()
            nc.tensor.transpose(PT3[0:4, 0:P], HT0[:, 0:4], id_s[:])
            out_sb = spool.tile([4, P], F32, name="t", tag="outsb")
            nc.scalar.copy(out_sb[:], PT3[0:4, 0:P])
            nc.sync.dma_start(
                out_d[0, :].rearrange("(a b) -> a b", a=4), out_sb[:])

    nc.compile()
    return nc


_NC_CACHE = None


def kernel(inputs, Wx, bx, Ws, bs, Wf, bf, children):
    global LAST_RESULT, _NC_CACHE
    inputs = np.asarray(inputs, np.float32)
    Wx = np.asarray(Wx, np.float32)
    bx = np.asarray(bx, np.float32)
    Ws = np.asarray(Ws, np.float32)
    bs = np.asarray(bs, np.float32)
    Wf = np.asarray(Wf, np.float32)
    bf = np.asarray(bf, np.float32)

    M2 = MEM
    # gate order (i, u, o, f); natural Wx order (i, f, o, u), Ws (i, o, u)
    Wxp = np.concatenate(
        [Wx[:, 0:M2], Wx[:, 3 * M2:4 * M2], Wx[:, 2 * M2:3 * M2], Wx[:, M2:2 * M2]], 1)
    Wsp = np.concatenate([Ws[:, 0:M2], Ws[:, 2 * M2:3 * M2], Ws[:, M2:2 * M2]], 1)
    bxp = np.concatenate([bx[0:M2], bx[3 * M2:4 * M2], bx[2 * M2:3 * M2], bx[M2:2 * M2]])
    bsp = np.concatenate([bs[0:M2], bs[2 * M2:3 * M2], bs[M2:2 * M2]])
    brow = np.zeros(4 * M2, np.float32)
    brow[0:3 * M2] = bxp[0:3 * M2] + bsp
    brow[3 * M2:] = bxp[3 * M2:] + bf
    bcol = np.ascontiguousarray(brow.reshape(16, P).T)

    def kmaj(a):
        # [512, F] -> [128, 4*F]  (k-major: col k*F + f)
        Fd = a.shape[1]
        return np.ascontiguousarray(
            a.reshape(KT, P, Fd).transpose(1, 0, 2).reshape(P, KT * Fd))

    Wx16 = Wxp.astype(np.float16)
    wxiu = kmaj(Wx16[:, 0:1024])
    wxof = kmaj(Wx16[:, 1024:2048])
    ws16 = kmaj(Wsp.astype(np.float16))
    wf16 = kmaj(Wf.astype(np.float16))
    ident = np.eye(P, dtype=np.float32)

    in_maps = []
    for c in range(NCORES):
        heaps = _core_heaps(c)
        valid = (heaps >= 0) & (heaps < N)
        M = np.zeros((NSLOT, IN_DIM), np.float32)
        M[valid] = inputs[N - 1 - heaps[valid]]
        xin = kmaj(np.ascontiguousarray(M.T).astype(np.float16))
        cmask = np.ascontiguousarray(
            np.tile(valid[:NL3].astype(np.float16)[None, :], (P, 1)))
        in_maps.append({
            "xin": xin, "wxiu": wxiu, "wxof": wxof, "ws": ws16, "wf": wf16,
            "bcol": bcol, "cmask": cmask, "ident": ident,
        })

    if _NC_CACHE is None:
        _NC_CACHE = _build_program()
    nc = _NC_CACHE

    res = run_bass_kernel_spmd(
        nc, in_maps, list(range(NCORES)),
        trace=bool(os.environ.get("BASS_TRACE")),
    )
    LAST_RESULT = res
    return np.ascontiguousarray(res.results[0]["out"])
